# revision 18
# baseline (speedup 1.0000x reference)
"""Multi-head attention TRN2 kernel, 8-core SPMD, head-pair pipelined.

Sharding: each core owns 1024 query rows (batch b = core//2, sequence half
core%2) and computes the full forward pass for those rows. No collectives.

Single-core structure: the 16 heads are processed as 8 head-pairs. For each
pair the kernel projects K/V (full 2048-key sequence) and Q (1024 rows) for
just that pair's 128 features, then runs attention for its two heads. The
projection matmuls of pair hp+1 are injected into the attention kc-loop of
pair hp, so the tensor engine's projection work fills the gaps left while the
scalar engine (exp, the per-core floor at ~266us) processes score chunks.

All matmul operands are bf16 (f32 PSUM accumulation). Score chunks are
[128 k, 1024 q] in PSUM; exp runs on the scalar engine (scale 1/8, no max
subtraction -- scores/8 ~ N(0,1)), writing bf16; the 0/1 mask multiplies on
the vector engine (all-bf16 SBUF operands hit the DVE 4x perf mode). PV runs
in flipped orientation -- lhsT = exp-chunk slice [128 k, 128 q], rhs = V
chunk [128 k, 65] (65th column of ones accumulates the softmax denominator)
-- producing X.T accumulators [128 q, 65] at full PE rate. Normalization is
then a per-partition scalar multiply; a PE transpose (identity matmul) turns
the normalized [128 q, 128 d] tiles back into [128 d, 1024 q] slabs for the
output projection, which runs as a tail phase.

PSUM budget (8 banks): a shared 2-deep ring of [128,1024]-f32 slots carries
score chunks AND projection partials (4 banks); two 2-deep rings of
[128,4,128]-f32 tiles carry the per-head PV accumulators (4 banks); the
transpose output borrows a PV-ring slot between heads.
"""

from contextlib import ExitStack

import numpy as np

B, S, D, H, DH = 4, 2048, 1024, 16, 64
NQ = 1024          # query rows per core
NK = 2048          # keys per core (full sequence of its batch)
NCORES = 8
NP = 8             # head pairs

_CACHE = {}


def _build():
    import concourse.mybir as mybir
    import concourse.tile as tile
    from concourse import bacc

    f32 = mybir.dt.float32
    bf16 = mybir.dt.bfloat16
    EXP = mybir.ActivationFunctionType.Exp

    nc = bacc.Bacc(
        "TRN2",
        target_bir_lowering=False,
        debug=False,
        enable_asserts=False,
        num_devices=NCORES,
    )

    xq_d = nc.dram_tensor("xq_t", [D, NQ], bf16, kind="ExternalInput").ap()
    xk_d = nc.dram_tensor("xk_t", [D, NK], bf16, kind="ExternalInput").ap()
    xv_d = nc.dram_tensor("xv_t", [D, NK], bf16, kind="ExternalInput").ap()
    wq_d = nc.dram_tensor("wq_t", [NP, D, 128], bf16, kind="ExternalInput").ap()
    wk_d = nc.dram_tensor("wk_t", [NP, D, 128], bf16, kind="ExternalInput").ap()
    wv_d = nc.dram_tensor("wv_t", [NP, D, 128], bf16, kind="ExternalInput").ap()
    wo_d = nc.dram_tensor("wo_t", [8, 8, 128, 128], bf16, kind="ExternalInput").ap()
    mk_d = nc.dram_tensor("mask_t", [NK, NQ], bf16, kind="ExternalInput").ap()
    id_d = nc.dram_tensor("ident", [128, 128], bf16, kind="ExternalInput").ap()
    wbc_d = nc.dram_tensor("wb_cols", [128, 24], f32, kind="ExternalInput").ap()
    out_d = nc.dram_tensor("out_t", [D, NQ], bf16, kind="ExternalOutput").ap()

    def split8(ap_2d):
        # [1024, N] dram view -> [128, 8, N]: partition p, chunk i, col
        return ap_2d.rearrange("(i p) q -> p i q", p=128)

    with tile.TileContext(nc) as tc:
        stk = ExitStack()

        kpool = stk.enter_context(tc.tile_pool(name="konst", bufs=1))
        wbc = kpool.tile([128, 24], f32, name="wbc")
        nc.sync.dma_start(wbc[:], wbc_d[:, :])
        ident = kpool.tile([128, 128], bf16, name="ident")
        nc.sync.dma_start(ident[:], id_d[:, :])

        # The cost model serializes all DMA transfers on one global device in
        # issue order, and the gpsimd SWDGE path is much costlier per byte --
        # so issue everything from the SP/Act hardware DGE queues, ordered by
        # when the data is first needed (K/Q proj first, then V and the mask
        # chunks which are consumed progressively).
        rpool = stk.enter_context(tc.tile_pool(name="resid", bufs=1))
        xk_sb = rpool.tile([128, 8, NK], bf16, name="xk_sb")
        for i in range(8):
            nc.sync.dma_start(xk_sb[:, i, :], xk_d[i * 128:(i + 1) * 128, :])
        xq_sb = rpool.tile([128, 8, NQ], bf16, name="xq_sb")
        for i in range(8):
            nc.sync.dma_start(xq_sb[:, i, :], xq_d[i * 128:(i + 1) * 128, :])
        xv_sb = rpool.tile([128, 8, NK], bf16, name="xv_sb")
        mask_sb = rpool.tile([128, 16, NQ], bf16, name="mask_sb")
        for i in range(8):
            nc.sync.dma_start(mask_sb[:, i, :], mk_d[i * 128:(i + 1) * 128, :])
            nc.sync.dma_start(xv_sb[:, i, :], xv_d[i * 128:(i + 1) * 128, :])
        for i in range(8, 16):
            nc.sync.dma_start(mask_sb[:, i, :], mk_d[i * 128:(i + 1) * 128, :])
        x_all = rpool.tile([128, 8, 8, 128], bf16, name="x_all")

        astk = ExitStack()
        wpool = astk.enter_context(tc.tile_pool(name="wring", bufs=2))
        kqpool = astk.enter_context(tc.tile_pool(name="kqring", bufs=2))
        vpool = astk.enter_context(tc.tile_pool(name="vring", bufs=2))
        pepool = astk.enter_context(tc.tile_pool(name="pering", bufs=4))
        xnpool = astk.enter_context(tc.tile_pool(name="xnring", bufs=2))
        rcpool = astk.enter_context(tc.tile_pool(name="rcring", bufs=2))
        stps = astk.enter_context(tc.tile_pool(name="stps", bufs=2, space="PSUM"))
        xtps = astk.enter_context(tc.tile_pool(name="xtps", bufs=2, space="PSUM"))

        slabs = {}

        def make_units(hp):
            """Projection work for pair hp as a list of small closures; each
            emits ~0.5us of PE work (plus its DMA / consumer ops)."""
            state = {}

            def dma_wv():
                w = wpool.tile([128, 8, 128], bf16, tag="wv", name="wv_sb")
                nc.sync.dma_start(w[:], split8(wv_d[hp, :, :]))
                state["wv"] = w
                v = vpool.tile([128, 16, 2, 65], bf16, tag="v", name="v_slab")
                nc.gpsimd.memset(v[:, :, :, 64:65], 1.0)
                state["v"] = v

            def v_unit(kb):
                def run():
                    ps = stps.tile([128, 128], f32, tag="ps", name="ps_v")
                    for i in range(8):
                        nc.tensor.matmul(
                            ps[:],
                            lhsT=xv_sb[:, i, kb * 128:(kb + 1) * 128],
                            rhs=state["wv"][:, i, :],
                            start=(i == 0),
                            stop=(i == 7),
                        )
                    nc.gpsimd.tensor_copy(
                        state["v"][:, kb, :, 0:64],
                        ps[:].rearrange("p (h e) -> p h e", e=64),
                    )
                return run

            def dma_wk():
                w = wpool.tile([128, 8, 128], bf16, tag="wk", name="wk_sb")
                nc.sync.dma_start(w[:], split8(wk_d[hp, :, :]))
                state["wk"] = w
                state["k"] = kqpool.tile([128, NK], bf16, tag="k", name="k_slab")

            def k_unit(kq, half):
                def run():
                    if half == 0:
                        state[f"psk{kq}"] = stps.tile(
                            [128, 512], f32, tag="ps", name="ps_k"
                        )
                    ps = state[f"psk{kq}"]
                    for i in range(4 * half, 4 * half + 4):
                        nc.tensor.matmul(
                            ps[:],
                            lhsT=state["wk"][:, i, :],
                            rhs=xk_sb[:, i, kq * 512:(kq + 1) * 512],
                            start=(i == 0),
                            stop=(i == 7),
                        )
                    if half == 1:
                        nc.vector.tensor_scalar_add(
                            state["k"][:, kq * 512:(kq + 1) * 512],
                            ps[:],
                            wbc[:, 8 + hp:9 + hp],
                        )
                return run

            def dma_wq():
                w = wpool.tile([128, 8, 128], bf16, tag="wq", name="wq_sb")
                nc.sync.dma_start(w[:], split8(wq_d[hp, :, :]))
                state["wq"] = w
                state["q"] = kqpool.tile([128, NQ], bf16, tag="q", name="q_slab")

            def q_unit(qh, half):
                def run():
                    if half == 0:
                        state[f"psq{qh}"] = stps.tile(
                            [128, 512], f32, tag="ps", name="ps_q"
                        )
                    ps = state[f"psq{qh}"]
                    for i in range(4 * half, 4 * half + 4):
                        nc.tensor.matmul(
                            ps[:],
                            lhsT=state["wq"][:, i, :],
                            rhs=xq_sb[:, i, qh * 512:(qh + 1) * 512],
                            start=(i == 0),
                            stop=(i == 7),
                        )
                    if half == 1:
                        nc.vector.tensor_scalar_add(
                            state["q"][:, qh * 512:(qh + 1) * 512],
                            ps[:],
                            wbc[:, hp:hp + 1],
                        )
                return run

            units = [dma_wk]
            units += [k_unit(kq, half) for kq in range(4) for half in range(2)]
            units += [dma_wq]
            units += [q_unit(qh, half) for qh in range(2) for half in range(2)]
            units += [dma_wv]
            units += [v_unit(kb) for kb in range(16)]
            slabs[hp] = state
            return units

        # Fill: project pair 0's K/Q and the first V chunks outright; the
        # remaining units flow through the global injection queue.
        from collections import deque

        inject = deque()
        u0 = make_units(0)
        for u in u0[:19]:  # dma_wk, K*8, dma_wq, Q*4, dma_wv, V*4
            u()
        inject.extend(u0[19:])

        for hp in range(NP):
            if hp + 1 < NP:
                inject.extend(make_units(hp + 1))
            st_hist = {}
            pe_hist = {}
            xn = xnpool.tile([128, 8, 128], bf16, tag="xn", name="xn")
            for hh in range(2):
                kslab = slabs[hp]["k"]
                qslab = slabs[hp]["q"]
                vslab = slabs[hp]["v"]
                ks = kslab[hh * 64:(hh + 1) * 64, :]
                qs = qslab[hh * 64:(hh + 1) * 64, :]
                xt_lo = xtps.tile(
                    [128, 4, 128], f32, tag="xt_lo", name="xt_lo", bufs=1
                )
                xt_hi = xtps.tile(
                    [128, 4, 128], f32, tag="xt_hi", name="xt_hi", bufs=1
                )
                xts = [xt_lo, xt_hi]
                for t in range(19):
                    if t < 16:
                        kc = t
                        st = stps.tile([128, NQ], f32, tag="st", name="st")
                        for qh in range(2):
                            nc.tensor.matmul(
                                st[:, qh * 512:(qh + 1) * 512],
                                lhsT=ks[:, kc * 128:(kc + 1) * 128],
                                rhs=qs[:, qh * 512:(qh + 1) * 512],
                                start=True,
                                stop=True,
                            )
                        if inject:
                            inject.popleft()()
                        pe = pepool.tile([128, NQ], bf16, tag="pe", name="pe")
                        nc.scalar.activation(pe[:], st[:], EXP, scale=0.125)
                        nc.vector.tensor_mul(pe[:], pe[:], mask_sb[:, kc, :])
                        pe_hist[kc] = pe
                    if t >= 3:
                        kc = t - 3
                        pe = pe_hist.pop(kc)
                        for qc in range(8):
                            nc.tensor.matmul(
                                xts[qc // 4][:, qc % 4, 0:65],
                                lhsT=pe[:, qc * 128:(qc + 1) * 128],
                                rhs=vslab[:, kc, hh, :],
                                start=(kc == 0),
                                stop=(kc == 15),
                            )
                rc = rcpool.tile([128, 8, 1], f32, tag="rc", name="rc")
                nc.vector.reciprocal(rc[:, 0:4, :], xt_lo[:, :, 64:65])
                nc.vector.reciprocal(rc[:, 4:8, :], xt_hi[:, :, 64:65])
                for qc in range(8):
                    nc.vector.tensor_scalar_mul(
                        xn[:, qc:qc + 1, hh * 64:(hh + 1) * 64],
                        xts[qc // 4][:, qc % 4:qc % 4 + 1, 0:64],
                        rc[:, qc:qc + 1, :],
                    )
            # Pair epilogue: transpose X.T tiles into [d, q] slabs.
            tp = stps.tile([128, 8, 128], bf16, tag="ps", name="tp")
            for qc in range(8):
                nc.tensor.transpose(tp[:, qc, :], xn[:, qc, :], ident[:])
            nc.vector.tensor_copy(x_all[:, hp], tp[:])
        astk.close()

        # Tail: output projection out.T[f, q] = sum_d Wo.T[d, f] * X[d, q].
        with (
            tc.tile_pool(name="wo_ring", bufs=3) as wopool,
            tc.tile_pool(name="oc_ring", bufs=3) as ocpool,
            tc.tile_pool(name="ops", bufs=2, space="PSUM") as opool,
        ):
            for fb in range(8):
                wo_sb = wopool.tile([128, 8, 128], bf16, tag="wo", name="wo_sb")
                nc.sync.dma_start(
                    wo_sb[:], wo_d[fb].rearrange("i p f -> p i f")
                )
                for qh in range(2):
                    op = opool.tile([128, 512], f32, tag="op", name="op")
                    for dp in range(8):
                        nc.tensor.matmul(
                            op[:],
                            lhsT=wo_sb[:, dp, :],
                            rhs=x_all[:, dp, qh * 4:(qh + 1) * 4, :],
                            start=(dp == 0),
                            stop=(dp == 7),
                        )
                    outc = ocpool.tile([128, 512], bf16, tag="oc", name="outc")
                    nc.vector.tensor_scalar_add(
                        outc[:], op[:], wbc[:, 16 + fb:17 + fb]
                    )
                    nc.sync.dma_start(
                        out_d[fb * 128:(fb + 1) * 128, qh * 512:(qh + 1) * 512],
                        outc[:],
                    )
        stk.close()

    nc.compile()
    return nc


def _get_nc():
    if "nc" not in _CACHE:
        _CACHE["nc"] = _build()
    return _CACHE["nc"]


def _prep(query, key, value, mask, Wq, bq, Wk, bk, Wv, bv, Wo, bo):
    import ml_dtypes

    f = np.float32
    bf = ml_dtypes.bfloat16

    def wt_tiles(W):  # W [D, D] -> [8, D, 128] slices of W.T along fout
        WT = np.ascontiguousarray(np.asarray(W, dtype=f).T)
        return np.ascontiguousarray(
            np.stack([WT[:, i * 128:(i + 1) * 128] for i in range(8)], 0)
        ).astype(bf)

    wq_t = wt_tiles(Wq)
    wk_t = wt_tiles(Wk)
    wv_t = wt_tiles(Wv)
    WoT = np.ascontiguousarray(np.asarray(Wo, dtype=f).T)
    wo_t = np.ascontiguousarray(
        np.stack(
            [
                np.stack(
                    [
                        WoT[dp * 128:(dp + 1) * 128, fb * 128:(fb + 1) * 128]
                        for dp in range(8)
                    ],
                    0,
                )
                for fb in range(8)
            ],
            0,
        )
    ).astype(bf)
    bo_eff = (
        np.asarray(bo, dtype=np.float64)
        + np.asarray(Wo, dtype=np.float64) @ np.asarray(bv, dtype=np.float64)
    ).astype(f)
    wb_cols = np.stack(
        [np.asarray(b).astype(f).reshape(8, 128).T for b in (bq, bk, bo_eff)],
        1,
    ).reshape(128, 24)
    wb_cols = np.ascontiguousarray(wb_cols)
    ident = np.eye(128, dtype=f).astype(bf)
    m2 = np.asarray(mask)[0, 0]  # [S, S] int
    in_maps = []
    for c in range(NCORES):
        b, half = c // 2, c % 2
        qsl = slice(half * NQ, (half + 1) * NQ)
        in_maps.append(
            {
                "xq_t": np.ascontiguousarray(
                    np.asarray(query)[b, qsl].T.astype(bf)
                ),
                "xk_t": np.ascontiguousarray(np.asarray(key)[b].T.astype(bf)),
                "xv_t": np.ascontiguousarray(np.asarray(value)[b].T.astype(bf)),
                "wq_t": wq_t,
                "wk_t": wk_t,
                "wv_t": wv_t,
                "wo_t": wo_t,
                "wb_cols": wb_cols,
                "ident": ident,
                "mask_t": np.ascontiguousarray(m2[qsl, :].T).astype(bf),
            }
        )
    return in_maps


def kernel(**inputs):
    from concourse.bass_utils import run_bass_kernel_spmd

    np_inputs = {k: np.asarray(v) for k, v in inputs.items()}
    in_maps = _prep(**np_inputs)
    nc = _get_nc()
    res = run_bass_kernel_spmd(nc, in_maps, list(range(NCORES)))
    out = np.empty((B, S, D), np.float32)
    for c in range(NCORES):
        b, half = c // 2, c % 2
        out[b, half * NQ:(half + 1) * NQ, :] = (
            res.results[c]["out_t"].astype(np.float32).T
        )
    return out


# revision 21
# speedup vs baseline: 1.0119x; 1.0119x over previous
"""Multi-head attention TRN2 kernel, 8-core SPMD, head-pair pipelined.

Sharding: each core owns 1024 query rows (batch b = core//2, sequence half
core%2) and computes the full forward pass for those rows. No collectives.

Single-core structure: the 16 heads are processed as 8 head-pairs. For each
pair the kernel projects K/V (full 2048-key sequence) and Q (1024 rows) for
just that pair's 128 features, then runs attention for its two heads. The
projection matmuls of pair hp+1 are injected into the attention kc-loop of
pair hp, so the tensor engine's projection work fills the gaps left while the
scalar engine (exp, the per-core floor at ~266us) processes score chunks.

All matmul operands are bf16 (f32 PSUM accumulation). Score chunks are
[128 k, 1024 q] in PSUM; exp runs on the scalar engine (scale 1/8, no max
subtraction -- scores/8 ~ N(0,1)), writing bf16; the 0/1 mask multiplies on
the vector engine (all-bf16 SBUF operands hit the DVE 4x perf mode). PV runs
in flipped orientation -- lhsT = exp-chunk slice [128 k, 128 q], rhs = V
chunk [128 k, 65] (65th column of ones accumulates the softmax denominator)
-- producing X.T accumulators [128 q, 65] at full PE rate. Normalization is
then a per-partition scalar multiply; a PE transpose (identity matmul) turns
the normalized [128 q, 128 d] tiles back into [128 d, 1024 q] slabs for the
output projection, which runs as a tail phase.

PSUM budget (8 banks): a shared 2-deep ring of [128,1024]-f32 slots carries
score chunks AND projection partials (4 banks); two 2-deep rings of
[128,4,128]-f32 tiles carry the per-head PV accumulators (4 banks); the
transpose output borrows a PV-ring slot between heads.
"""

from contextlib import ExitStack

import numpy as np

B, S, D, H, DH = 4, 2048, 1024, 16, 64
NQ = 1024          # query rows per core
NK = 2048          # keys per core (full sequence of its batch)
NCORES = 8
NP = 8             # head pairs

_CACHE = {}


def _build():
    import concourse.mybir as mybir
    import concourse.tile as tile
    from concourse import bacc

    f32 = mybir.dt.float32
    bf16 = mybir.dt.bfloat16
    EXP = mybir.ActivationFunctionType.Exp

    nc = bacc.Bacc(
        "TRN2",
        target_bir_lowering=False,
        debug=False,
        enable_asserts=False,
        num_devices=NCORES,
    )

    xq_d = nc.dram_tensor("xq_t", [D, NQ], bf16, kind="ExternalInput").ap()
    xk_d = nc.dram_tensor("xk_t", [D, NK], bf16, kind="ExternalInput").ap()
    xv_d = nc.dram_tensor("xv_t", [D, NK], bf16, kind="ExternalInput").ap()
    wq_d = nc.dram_tensor("wq_t", [NP, D, 128], bf16, kind="ExternalInput").ap()
    wk_d = nc.dram_tensor("wk_t", [NP, D, 128], bf16, kind="ExternalInput").ap()
    wv_d = nc.dram_tensor("wv_t", [NP, D, 128], bf16, kind="ExternalInput").ap()
    wo_d = nc.dram_tensor("wo_t", [8, 8, 128, 128], bf16, kind="ExternalInput").ap()
    mk_d = nc.dram_tensor("mask_t", [NK, NQ], bf16, kind="ExternalInput").ap()
    id_d = nc.dram_tensor("ident", [128, 128], bf16, kind="ExternalInput").ap()
    wbc_d = nc.dram_tensor("wb_cols", [128, 24], f32, kind="ExternalInput").ap()
    out_d = nc.dram_tensor("out_t", [D, NQ], bf16, kind="ExternalOutput").ap()

    def split8(ap_2d):
        # [1024, N] dram view -> [128, 8, N]: partition p, chunk i, col
        return ap_2d.rearrange("(i p) q -> p i q", p=128)

    with tile.TileContext(nc) as tc:
        stk = ExitStack()

        kpool = stk.enter_context(tc.tile_pool(name="konst", bufs=1))
        wbc = kpool.tile([128, 24], f32, name="wbc")
        nc.sync.dma_start(wbc[:], wbc_d[:, :])
        ident = kpool.tile([128, 128], bf16, name="ident")
        nc.sync.dma_start(ident[:], id_d[:, :])

        # The cost model serializes all DMA transfers on one global device in
        # issue order, and the gpsimd SWDGE path is much costlier per byte --
        # so issue everything from the SP/Act hardware DGE queues, ordered by
        # when the data is first needed (K/Q proj first, then V and the mask
        # chunks which are consumed progressively).
        rpool = stk.enter_context(tc.tile_pool(name="resid", bufs=1))
        xk_sb = rpool.tile([128, 8, NK], bf16, name="xk_sb")
        for i in range(8):
            nc.sync.dma_start(xk_sb[:, i, :], xk_d[i * 128:(i + 1) * 128, :])
        xq_sb = rpool.tile([128, 8, NQ], bf16, name="xq_sb")
        for i in range(8):
            nc.sync.dma_start(xq_sb[:, i, :], xq_d[i * 128:(i + 1) * 128, :])
        xv_sb = rpool.tile([128, 8, NK], bf16, name="xv_sb")
        for i in range(8):
            nc.sync.dma_start(xv_sb[:, i, :], xv_d[i * 128:(i + 1) * 128, :])
        mask_sb = rpool.tile([128, 16, NQ], bf16, name="mask_sb")
        for i in range(16):
            nc.sync.dma_start(mask_sb[:, i, :], mk_d[i * 128:(i + 1) * 128, :])
        x_all = rpool.tile([128, 8, 8, 128], bf16, name="x_all")

        astk = ExitStack()
        wpool = astk.enter_context(tc.tile_pool(name="wring", bufs=2))
        kqpool = astk.enter_context(tc.tile_pool(name="kqring", bufs=2))
        vpool = astk.enter_context(tc.tile_pool(name="vring", bufs=2))
        pepool = astk.enter_context(tc.tile_pool(name="pering", bufs=12))
        xnpool = astk.enter_context(tc.tile_pool(name="xnring", bufs=2))
        rcpool = astk.enter_context(tc.tile_pool(name="rcring", bufs=2))
        stps = astk.enter_context(tc.tile_pool(name="stps", bufs=2, space="PSUM"))
        xtps = astk.enter_context(tc.tile_pool(name="xtps", bufs=2, space="PSUM"))

        slabs = {}

        def make_units(hp):
            """Projection work for pair hp as a list of small closures; each
            emits ~0.5us of PE work (plus its DMA / consumer ops)."""
            state = {}

            def dma_wv():
                w = wpool.tile([128, 8, 128], bf16, tag="wv", name="wv_sb")
                nc.sync.dma_start(w[:], split8(wv_d[hp, :, :]))
                state["wv"] = w
                v = vpool.tile([128, 16, 2, 65], bf16, tag="v", name="v_slab")
                nc.gpsimd.memset(v[:, :, :, 64:65], 1.0)
                state["v"] = v

            def v_unit(kb):
                def run():
                    ps = stps.tile([128, 128], f32, tag="ps", name="ps_v")
                    for i in range(8):
                        nc.tensor.matmul(
                            ps[:],
                            lhsT=xv_sb[:, i, kb * 128:(kb + 1) * 128],
                            rhs=state["wv"][:, i, :],
                            start=(i == 0),
                            stop=(i == 7),
                        )
                    nc.gpsimd.tensor_copy(
                        state["v"][:, kb, :, 0:64],
                        ps[:].rearrange("p (h e) -> p h e", e=64),
                    )
                return run

            def dma_wk():
                w = wpool.tile([128, 8, 128], bf16, tag="wk", name="wk_sb")
                nc.sync.dma_start(w[:], split8(wk_d[hp, :, :]))
                state["wk"] = w
                state["k"] = kqpool.tile([128, NK], bf16, tag="k", name="k_slab")

            def k_unit(kq, half):
                def run():
                    if half == 0:
                        state[f"psk{kq}"] = stps.tile(
                            [128, 512], f32, tag="ps", name="ps_k"
                        )
                    ps = state[f"psk{kq}"]
                    for i in range(4 * half, 4 * half + 4):
                        nc.tensor.matmul(
                            ps[:],
                            lhsT=state["wk"][:, i, :],
                            rhs=xk_sb[:, i, kq * 512:(kq + 1) * 512],
                            start=(i == 0),
                            stop=(i == 7),
                        )
                    if half == 1:
                        nc.vector.tensor_scalar_add(
                            state["k"][:, kq * 512:(kq + 1) * 512],
                            ps[:],
                            wbc[:, 8 + hp:9 + hp],
                        )
                return run

            def dma_wq():
                w = wpool.tile([128, 8, 128], bf16, tag="wq", name="wq_sb")
                nc.sync.dma_start(w[:], split8(wq_d[hp, :, :]))
                state["wq"] = w
                state["q"] = kqpool.tile([128, NQ], bf16, tag="q", name="q_slab")

            def q_unit(qh, half):
                def run():
                    if half == 0:
                        state[f"psq{qh}"] = stps.tile(
                            [128, 512], f32, tag="ps", name="ps_q"
                        )
                    ps = state[f"psq{qh}"]
                    for i in range(4 * half, 4 * half + 4):
                        nc.tensor.matmul(
                            ps[:],
                            lhsT=state["wq"][:, i, :],
                            rhs=xq_sb[:, i, qh * 512:(qh + 1) * 512],
                            start=(i == 0),
                            stop=(i == 7),
                        )
                    if half == 1:
                        nc.vector.tensor_scalar_add(
                            state["q"][:, qh * 512:(qh + 1) * 512],
                            ps[:],
                            wbc[:, hp:hp + 1],
                        )
                return run

            units = [dma_wk]
            units += [k_unit(kq, half) for kq in range(4) for half in range(2)]
            units += [dma_wq]
            units += [q_unit(qh, half) for qh in range(2) for half in range(2)]
            units += [dma_wv]
            units += [v_unit(kb) for kb in range(16)]
            slabs[hp] = state
            return units

        # One global software pipeline over all 256 score chunks (16 heads x
        # 16 key-chunks): scores(c) / exp(c) / mask(c) stream ahead while
        # PV(c - LAG) trails, so head and pair boundaries never drain the
        # scalar engine. Projection units for pair hp+1 flow through the
        # injection queue, one per chunk slot.
        from collections import deque

        LAG = 10
        CH = H * 16

        inject = deque()
        u0 = make_units(0)
        for u in u0[:14]:  # dma_wk, K*8, dma_wq, Q*4
            u()
        inject.extend(u0[14:])  # dma_wv, V*16

        pe_hist = {}
        xts_of = {}
        xn_of = {}

        for c in range(CH + LAG):
            if c < CH:
                head, kc = c // 16, c % 16
                hp, hh = head // 2, head % 2
                if kc == 0 and hh == 0 and hp + 1 < NP:
                    inject.extend(make_units(hp + 1))
                ks = slabs[hp]["k"][hh * 64:(hh + 1) * 64, :]
                qs = slabs[hp]["q"][hh * 64:(hh + 1) * 64, :]
                st = stps.tile([128, NQ], f32, tag="st", name="st")
                for qh in range(2):
                    nc.tensor.matmul(
                        st[:, qh * 512:(qh + 1) * 512],
                        lhsT=ks[:, kc * 128:(kc + 1) * 128],
                        rhs=qs[:, qh * 512:(qh + 1) * 512],
                        start=True,
                        stop=True,
                    )
                if inject:
                    inject.popleft()()
                pe = pepool.tile([128, NQ], bf16, tag="pe", name="pe")
                nc.scalar.activation(pe[:], st[:], EXP, scale=0.125)
                nc.vector.tensor_mul(pe[:], pe[:], mask_sb[:, kc, :])
                pe_hist[c] = pe
            if c >= LAG:
                cc = c - LAG
                head, kc = cc // 16, cc % 16
                hp, hh = head // 2, head % 2
                pe = pe_hist.pop(cc)
                if kc == 0:
                    xts_of[head] = (
                        xtps.tile([128, 4, 128], f32, tag="xt_lo",
                                  name="xt_lo", bufs=1),
                        xtps.tile([128, 4, 128], f32, tag="xt_hi",
                                  name="xt_hi", bufs=1),
                    )
                    if hh == 0:
                        xn_of[hp] = xnpool.tile(
                            [128, 8, 128], bf16, tag="xn", name="xn"
                        )
                xts = xts_of[head]
                for qc in range(8):
                    nc.tensor.matmul(
                        xts[qc // 4][:, qc % 4, 0:65],
                        lhsT=pe[:, qc * 128:(qc + 1) * 128],
                        rhs=slabs[hp]["v"][:, kc, hh, :],
                        start=(kc == 0),
                        stop=(kc == 15),
                    )
                if kc == 15:
                    xt_lo, xt_hi = xts_of.pop(head)
                    xn = xn_of[hp]
                    rc = rcpool.tile([128, 8, 1], f32, tag="rc", name="rc")
                    nc.vector.reciprocal(rc[:, 0:4, :], xt_lo[:, :, 64:65])
                    nc.vector.reciprocal(rc[:, 4:8, :], xt_hi[:, :, 64:65])
                    for qc in range(8):
                        nc.vector.tensor_scalar_mul(
                            xn[:, qc:qc + 1, hh * 64:(hh + 1) * 64],
                            (xt_lo, xt_hi)[qc // 4][
                                :, qc % 4:qc % 4 + 1, 0:64
                            ],
                            rc[:, qc:qc + 1, :],
                        )
                    if hh == 1:
                        xn_of.pop(hp)
                        tp = stps.tile([128, 8, 128], bf16, tag="ps", name="tp")
                        for qc in range(8):
                            nc.tensor.transpose(
                                tp[:, qc, :], xn[:, qc, :], ident[:]
                            )
                        nc.vector.tensor_copy(x_all[:, hp], tp[:])
        astk.close()

        # Tail: output projection out.T[f, q] = sum_d Wo.T[d, f] * X[d, q].
        with (
            tc.tile_pool(name="wo_ring", bufs=3) as wopool,
            tc.tile_pool(name="oc_ring", bufs=3) as ocpool,
            tc.tile_pool(name="ops", bufs=2, space="PSUM") as opool,
        ):
            for fb in range(8):
                wo_sb = wopool.tile([128, 8, 128], bf16, tag="wo", name="wo_sb")
                nc.sync.dma_start(
                    wo_sb[:], wo_d[fb].rearrange("i p f -> p i f")
                )
                for qh in range(2):
                    op = opool.tile([128, 512], f32, tag="op", name="op")
                    for dp in range(8):
                        nc.tensor.matmul(
                            op[:],
                            lhsT=wo_sb[:, dp, :],
                            rhs=x_all[:, dp, qh * 4:(qh + 1) * 4, :],
                            start=(dp == 0),
                            stop=(dp == 7),
                        )
                    outc = ocpool.tile([128, 512], bf16, tag="oc", name="outc")
                    nc.vector.tensor_scalar_add(
                        outc[:], op[:], wbc[:, 16 + fb:17 + fb]
                    )
                    nc.sync.dma_start(
                        out_d[fb * 128:(fb + 1) * 128, qh * 512:(qh + 1) * 512],
                        outc[:],
                    )
        stk.close()

    nc.compile()
    return nc


def _get_nc():
    if "nc" not in _CACHE:
        _CACHE["nc"] = _build()
    return _CACHE["nc"]


def _prep(query, key, value, mask, Wq, bq, Wk, bk, Wv, bv, Wo, bo):
    import ml_dtypes

    f = np.float32
    bf = ml_dtypes.bfloat16

    def wt_tiles(W):  # W [D, D] -> [8, D, 128] slices of W.T along fout
        WT = np.ascontiguousarray(np.asarray(W, dtype=f).T)
        return np.ascontiguousarray(
            np.stack([WT[:, i * 128:(i + 1) * 128] for i in range(8)], 0)
        ).astype(bf)

    wq_t = wt_tiles(Wq)
    wk_t = wt_tiles(Wk)
    wv_t = wt_tiles(Wv)
    WoT = np.ascontiguousarray(np.asarray(Wo, dtype=f).T)
    wo_t = np.ascontiguousarray(
        np.stack(
            [
                np.stack(
                    [
                        WoT[dp * 128:(dp + 1) * 128, fb * 128:(fb + 1) * 128]
                        for dp in range(8)
                    ],
                    0,
                )
                for fb in range(8)
            ],
            0,
        )
    ).astype(bf)
    bo_eff = (
        np.asarray(bo, dtype=np.float64)
        + np.asarray(Wo, dtype=np.float64) @ np.asarray(bv, dtype=np.float64)
    ).astype(f)
    wb_cols = np.stack(
        [np.asarray(b).astype(f).reshape(8, 128).T for b in (bq, bk, bo_eff)],
        1,
    ).reshape(128, 24)
    wb_cols = np.ascontiguousarray(wb_cols)
    ident = np.eye(128, dtype=f).astype(bf)
    m2 = np.asarray(mask)[0, 0]  # [S, S] int
    in_maps = []
    for c in range(NCORES):
        b, half = c // 2, c % 2
        qsl = slice(half * NQ, (half + 1) * NQ)
        in_maps.append(
            {
                "xq_t": np.ascontiguousarray(
                    np.asarray(query)[b, qsl].T.astype(bf)
                ),
                "xk_t": np.ascontiguousarray(np.asarray(key)[b].T.astype(bf)),
                "xv_t": np.ascontiguousarray(np.asarray(value)[b].T.astype(bf)),
                "wq_t": wq_t,
                "wk_t": wk_t,
                "wv_t": wv_t,
                "wo_t": wo_t,
                "wb_cols": wb_cols,
                "ident": ident,
                "mask_t": np.ascontiguousarray(m2[qsl, :].T).astype(bf),
            }
        )
    return in_maps


def kernel(**inputs):
    from concourse.bass_utils import run_bass_kernel_spmd

    np_inputs = {k: np.asarray(v) for k, v in inputs.items()}
    in_maps = _prep(**np_inputs)
    nc = _get_nc()
    res = run_bass_kernel_spmd(nc, in_maps, list(range(NCORES)))
    out = np.empty((B, S, D), np.float32)
    for c in range(NCORES):
        b, half = c // 2, c % 2
        out[b, half * NQ:(half + 1) * NQ, :] = (
            res.results[c]["out_t"].astype(np.float32).T
        )
    return out


# revision 31
# speedup vs baseline: 1.0153x; 1.0034x over previous
"""Multi-head attention TRN2 kernel, 8-core SPMD, head-pair pipelined.

Sharding: each core owns 1024 query rows (batch b = core//2, sequence half
core%2) and computes the full forward pass for those rows. No collectives.

Single-core structure: the 16 heads are processed as 8 head-pairs. For each
pair the kernel projects K/V (full 2048-key sequence) and Q (1024 rows) for
just that pair's 128 features, then runs attention for its two heads. The
projection matmuls of pair hp+1 are injected into the attention kc-loop of
pair hp, so the tensor engine's projection work fills the gaps left while the
scalar engine (exp, the per-core floor at ~266us) processes score chunks.

All matmul operands are bf16 (f32 PSUM accumulation). Score chunks are
[128 k, 1024 q] in PSUM; exp runs on the scalar engine (scale 1/8, no max
subtraction -- scores/8 ~ N(0,1)), writing bf16; the 0/1 mask multiplies on
the vector engine (all-bf16 SBUF operands hit the DVE 4x perf mode). PV runs
in flipped orientation -- lhsT = exp-chunk slice [128 k, 128 q], rhs = V
chunk [128 k, 65] (65th column of ones accumulates the softmax denominator)
-- producing X.T accumulators [128 q, 65] at full PE rate. Normalization is
then a per-partition scalar multiply; a PE transpose (identity matmul) turns
the normalized [128 q, 128 d] tiles back into [128 d, 1024 q] slabs for the
output projection, which runs as a tail phase.

PSUM budget (8 banks): a shared 2-deep ring of [128,1024]-f32 slots carries
score chunks AND projection partials (4 banks); two 2-deep rings of
[128,4,128]-f32 tiles carry the per-head PV accumulators (4 banks); the
transpose output borrows a PV-ring slot between heads.
"""

from contextlib import ExitStack

import numpy as np

B, S, D, H, DH = 4, 2048, 1024, 16, 64
NQ = 1024          # query rows per core
NK = 2048          # keys per core (full sequence of its batch)
NCORES = 8
NP = 8             # head pairs

_CACHE = {}


def _build():
    import concourse.mybir as mybir
    import concourse.tile as tile
    from concourse import bacc

    f32 = mybir.dt.float32
    bf16 = mybir.dt.bfloat16
    EXP = mybir.ActivationFunctionType.Exp

    nc = bacc.Bacc(
        "TRN2",
        target_bir_lowering=False,
        debug=False,
        enable_asserts=False,
        num_devices=NCORES,
    )

    xq_d = nc.dram_tensor("xq_t", [D, NQ], bf16, kind="ExternalInput").ap()
    xk_d = nc.dram_tensor("xk_t", [D, NK], bf16, kind="ExternalInput").ap()
    xv_d = nc.dram_tensor("xv_t", [D, NK], bf16, kind="ExternalInput").ap()
    wq_d = nc.dram_tensor("wq_t", [NP, D, 128], bf16, kind="ExternalInput").ap()
    wk_d = nc.dram_tensor("wk_t", [NP, D, 128], bf16, kind="ExternalInput").ap()
    wv_d = nc.dram_tensor("wv_t", [NP, D, 128], bf16, kind="ExternalInput").ap()
    wo_d = nc.dram_tensor("wo_t", [8, 8, 128, 128], bf16, kind="ExternalInput").ap()
    mk_d = nc.dram_tensor("mask_t", [NK, NQ], bf16, kind="ExternalInput").ap()
    id_d = nc.dram_tensor("ident", [128, 128], bf16, kind="ExternalInput").ap()
    wbc_d = nc.dram_tensor("wb_cols", [128, 24], f32, kind="ExternalInput").ap()
    out_d = nc.dram_tensor("out_t", [D, NQ], bf16, kind="ExternalOutput").ap()

    def split8(ap_2d):
        # [1024, N] dram view -> [128, 8, N]: partition p, chunk i, col
        return ap_2d.rearrange("(i p) q -> p i q", p=128)

    with tile.TileContext(nc) as tc:
        stk = ExitStack()

        kpool = stk.enter_context(tc.tile_pool(name="konst", bufs=1))
        wbc = kpool.tile([128, 24], f32, name="wbc")
        nc.sync.dma_start(wbc[:], wbc_d[:, :])
        ident = kpool.tile([128, 128], bf16, name="ident")
        nc.sync.dma_start(ident[:], id_d[:, :])

        # The cost model serializes all DMA transfers on one global device in
        # issue order, and the gpsimd SWDGE path is much costlier per byte --
        # so issue everything from the SP/Act hardware DGE queues, ordered by
        # when the data is first needed (K/Q proj first, then V and the mask
        # chunks which are consumed progressively).
        rpool = stk.enter_context(tc.tile_pool(name="resid", bufs=1))
        xk_sb = rpool.tile([128, 8, NK], bf16, name="xk_sb")
        for i in range(8):
            nc.sync.dma_start(xk_sb[:, i, :], xk_d[i * 128:(i + 1) * 128, :])
        xq_sb = rpool.tile([128, 8, NQ], bf16, name="xq_sb")
        for i in range(8):
            nc.sync.dma_start(xq_sb[:, i, :], xq_d[i * 128:(i + 1) * 128, :])
        xv_sb = rpool.tile([128, 8, NK], bf16, name="xv_sb")
        for i in range(8):
            nc.sync.dma_start(xv_sb[:, i, :], xv_d[i * 128:(i + 1) * 128, :])
        mask_sb = rpool.tile([128, 16, NQ], bf16, name="mask_sb")
        for i in range(16):
            nc.sync.dma_start(mask_sb[:, i, :], mk_d[i * 128:(i + 1) * 128, :])
        x_all = rpool.tile([128, 8, 8, 128], bf16, name="x_all")

        astk = ExitStack()
        wpool = astk.enter_context(tc.tile_pool(name="wring", bufs=2))
        kqpool = astk.enter_context(tc.tile_pool(name="kqring", bufs=2))
        vpool = astk.enter_context(tc.tile_pool(name="vring", bufs=2))
        pepool = astk.enter_context(tc.tile_pool(name="pering", bufs=16))
        xnpool = astk.enter_context(tc.tile_pool(name="xnring", bufs=2))
        rcpool = astk.enter_context(tc.tile_pool(name="rcring", bufs=2))
        stps = astk.enter_context(tc.tile_pool(name="stps", bufs=2, space="PSUM"))
        xtps = astk.enter_context(tc.tile_pool(name="xtps", bufs=2, space="PSUM"))

        slabs = {}

        def make_units(hp):
            """Projection work for pair hp as a list of small closures; each
            emits ~0.5us of PE work (plus its DMA / consumer ops)."""
            state = {
                "k": kqpool.tile([128, NK], bf16, tag="k", name="k_slab"),
                "q": kqpool.tile([128, NQ], bf16, tag="q", name="q_slab"),
                "v": vpool.tile([128, 16, 2, 65], bf16, tag="v", name="v_slab"),
            }

            def dma_wv():
                w = wpool.tile([128, 8, 128], bf16, tag="wv", name="wv_sb")
                nc.sync.dma_start(w[:], split8(wv_d[hp, :, :]))
                state["wv"] = w
                nc.gpsimd.memset(state["v"][:, :, :, 64:65], 1.0)

            def v_unit(kb):
                def run():
                    ps = stps.tile([128, 128], f32, tag="ps", name="ps_v")
                    for i in range(8):
                        nc.tensor.matmul(
                            ps[:],
                            lhsT=xv_sb[:, i, kb * 128:(kb + 1) * 128],
                            rhs=state["wv"][:, i, :],
                            start=(i == 0),
                            stop=(i == 7),
                        )
                    nc.gpsimd.tensor_copy(
                        state["v"][:, kb, :, 0:64],
                        ps[:].rearrange("p (h e) -> p h e", e=64),
                    )
                return run

            def dma_wk():
                w = wpool.tile([128, 8, 128], bf16, tag="wk", name="wk_sb")
                nc.sync.dma_start(w[:], split8(wk_d[hp, :, :]))
                state["wk"] = w

            def k_unit(kq, half):
                def run():
                    if half == 0:
                        state[f"psk{kq}"] = stps.tile(
                            [128, 512], f32, tag="ps", name="ps_k"
                        )
                    ps = state[f"psk{kq}"]
                    for i in range(4 * half, 4 * half + 4):
                        nc.tensor.matmul(
                            ps[:],
                            lhsT=state["wk"][:, i, :],
                            rhs=xk_sb[:, i, kq * 512:(kq + 1) * 512],
                            start=(i == 0),
                            stop=(i == 7),
                        )
                    if half == 1:
                        nc.vector.tensor_scalar_add(
                            state["k"][:, kq * 512:(kq + 1) * 512],
                            ps[:],
                            wbc[:, 8 + hp:9 + hp],
                        )
                return run

            def dma_wq():
                w = wpool.tile([128, 8, 128], bf16, tag="wq", name="wq_sb")
                nc.sync.dma_start(w[:], split8(wq_d[hp, :, :]))
                state["wq"] = w

            def q_unit(qh, half):
                def run():
                    if half == 0:
                        state[f"psq{qh}"] = stps.tile(
                            [128, 512], f32, tag="ps", name="ps_q"
                        )
                    ps = state[f"psq{qh}"]
                    for i in range(4 * half, 4 * half + 4):
                        nc.tensor.matmul(
                            ps[:],
                            lhsT=state["wq"][:, i, :],
                            rhs=xq_sb[:, i, qh * 512:(qh + 1) * 512],
                            start=(i == 0),
                            stop=(i == 7),
                        )
                    if half == 1:
                        nc.vector.tensor_scalar_add(
                            state["q"][:, qh * 512:(qh + 1) * 512],
                            ps[:],
                            wbc[:, hp:hp + 1],
                        )
                return run

            units = [dma_wk]
            units += [k_unit(kq, half) for kq in range(4) for half in range(2)]
            units += [dma_wq]
            units += [q_unit(qh, half) for qh in range(2) for half in range(2)]
            units += [dma_wv]
            units += [v_unit(kb) for kb in range(16)]
            slabs[hp] = state
            return units

        # One global software pipeline over all 256 score chunks (16 heads x
        # 16 key-chunks): scores(c) / exp(c) / mask(c) stream ahead while
        # PV(c - LAG) trails, so head and pair boundaries never drain the
        # scalar engine. Projection units for pair hp+1 flow through the
        # injection queue, one per chunk slot.
        from collections import deque

        LAG = 14
        CH = H * 16

        inject = deque()
        u0 = make_units(0)
        for u in u0[:14]:  # dma_wk, K*8, dma_wq, Q*4
            u()
        # Pad before pair 0's V units so the PE does not park on them while
        # the xv load is still in flight (PE executes in order).
        inject.append(u0[14])  # dma_wv
        inject.extend([None] * 8)
        inject.extend(u0[15:])  # V*16

        pe_hist = {}
        xts_of = {}
        xn_of = {}

        for c in range(CH + LAG):
            if c < CH:
                head, kc = c // 16, c % 16
                hp, hh = head // 2, head % 2
                if kc == 0 and hh == 0 and hp + 1 < NP:
                    inject.extend(make_units(hp + 1))
                ks = slabs[hp]["k"][hh * 64:(hh + 1) * 64, :]
                qs = slabs[hp]["q"][hh * 64:(hh + 1) * 64, :]
                st = stps.tile([128, NQ], f32, tag="st", name="st")
                for qh in range(2):
                    nc.tensor.matmul(
                        st[:, qh * 512:(qh + 1) * 512],
                        lhsT=ks[:, kc * 128:(kc + 1) * 128],
                        rhs=qs[:, qh * 512:(qh + 1) * 512],
                        start=True,
                        stop=True,
                    )
                if inject:
                    u = inject.popleft()
                    if u is not None:
                        u()
                pe = pepool.tile([128, NQ], bf16, tag="pe", name="pe")
                nc.scalar.activation(pe[:], st[:], EXP, scale=0.125)
                nc.vector.tensor_mul(pe[:], pe[:], mask_sb[:, kc, :])
                pe_hist[c] = pe
            if c >= LAG:
                cc = c - LAG
                head, kc = cc // 16, cc % 16
                hp, hh = head // 2, head % 2
                pe = pe_hist.pop(cc)
                if kc == 0:
                    xts_of[head] = (
                        xtps.tile([128, 4, 128], f32, tag="xt_lo",
                                  name="xt_lo", bufs=1),
                        xtps.tile([128, 4, 128], f32, tag="xt_hi",
                                  name="xt_hi", bufs=1),
                    )
                    if hh == 0:
                        xn_of[hp] = xnpool.tile(
                            [128, 8, 128], bf16, tag="xn", name="xn"
                        )
                xts = xts_of[head]
                for qc in range(8):
                    nc.tensor.matmul(
                        xts[qc // 4][:, qc % 4, 0:65],
                        lhsT=pe[:, qc * 128:(qc + 1) * 128],
                        rhs=slabs[hp]["v"][:, kc, hh, :],
                        start=(kc == 0),
                        stop=(kc == 15),
                    )
                if kc == 15:
                    xt_lo, xt_hi = xts_of.pop(head)
                    xn = xn_of[hp]
                    rc = rcpool.tile([128, 8, 1], f32, tag="rc", name="rc")
                    nc.vector.reciprocal(rc[:, 0:4, :], xt_lo[:, :, 64:65])
                    nc.vector.reciprocal(rc[:, 4:8, :], xt_hi[:, :, 64:65])
                    for qc in range(8):
                        nc.vector.tensor_scalar_mul(
                            xn[:, qc:qc + 1, hh * 64:(hh + 1) * 64],
                            (xt_lo, xt_hi)[qc // 4][
                                :, qc % 4:qc % 4 + 1, 0:64
                            ],
                            rc[:, qc:qc + 1, :],
                        )
                    if hh == 1:
                        xn_of.pop(hp)
                        tp = stps.tile([128, 8, 128], bf16, tag="ps", name="tp")
                        for qc in range(8):
                            nc.tensor.transpose(
                                tp[:, qc, :], xn[:, qc, :], ident[:]
                            )
                        nc.vector.tensor_copy(x_all[:, hp], tp[:])
        astk.close()

        # Tail: output projection out.T[f, q] = sum_d Wo.T[d, f] * X[d, q].
        with (
            tc.tile_pool(name="wo_ring", bufs=3) as wopool,
            tc.tile_pool(name="oc_ring", bufs=3) as ocpool,
            tc.tile_pool(name="ops", bufs=2, space="PSUM") as opool,
        ):
            for fb in range(8):
                wo_sb = wopool.tile([128, 8, 128], bf16, tag="wo", name="wo_sb")
                nc.sync.dma_start(
                    wo_sb[:], wo_d[fb].rearrange("i p f -> p i f")
                )
                for qh in range(2):
                    op = opool.tile([128, 512], f32, tag="op", name="op")
                    for dp in range(8):
                        nc.tensor.matmul(
                            op[:],
                            lhsT=wo_sb[:, dp, :],
                            rhs=x_all[:, dp, qh * 4:(qh + 1) * 4, :],
                            start=(dp == 0),
                            stop=(dp == 7),
                        )
                    outc = ocpool.tile([128, 512], bf16, tag="oc", name="outc")
                    nc.vector.tensor_scalar_add(
                        outc[:], op[:], wbc[:, 16 + fb:17 + fb]
                    )
                    nc.sync.dma_start(
                        out_d[fb * 128:(fb + 1) * 128, qh * 512:(qh + 1) * 512],
                        outc[:],
                    )
        stk.close()

    nc.compile()
    return nc


def _get_nc():
    if "nc" not in _CACHE:
        _CACHE["nc"] = _build()
    return _CACHE["nc"]


def _prep(query, key, value, mask, Wq, bq, Wk, bk, Wv, bv, Wo, bo):
    import ml_dtypes

    f = np.float32
    bf = ml_dtypes.bfloat16

    def wt_tiles(W):  # W [D, D] -> [8, D, 128] slices of W.T along fout
        WT = np.ascontiguousarray(np.asarray(W, dtype=f).T)
        return np.ascontiguousarray(
            np.stack([WT[:, i * 128:(i + 1) * 128] for i in range(8)], 0)
        ).astype(bf)

    wq_t = wt_tiles(Wq)
    wk_t = wt_tiles(Wk)
    wv_t = wt_tiles(Wv)
    WoT = np.ascontiguousarray(np.asarray(Wo, dtype=f).T)
    wo_t = np.ascontiguousarray(
        np.stack(
            [
                np.stack(
                    [
                        WoT[dp * 128:(dp + 1) * 128, fb * 128:(fb + 1) * 128]
                        for dp in range(8)
                    ],
                    0,
                )
                for fb in range(8)
            ],
            0,
        )
    ).astype(bf)
    bo_eff = (
        np.asarray(bo, dtype=np.float64)
        + np.asarray(Wo, dtype=np.float64) @ np.asarray(bv, dtype=np.float64)
    ).astype(f)
    wb_cols = np.stack(
        [np.asarray(b).astype(f).reshape(8, 128).T for b in (bq, bk, bo_eff)],
        1,
    ).reshape(128, 24)
    wb_cols = np.ascontiguousarray(wb_cols)
    ident = np.eye(128, dtype=f).astype(bf)
    m2 = np.asarray(mask)[0, 0]  # [S, S] int
    in_maps = []
    for c in range(NCORES):
        b, half = c // 2, c % 2
        qsl = slice(half * NQ, (half + 1) * NQ)
        in_maps.append(
            {
                "xq_t": np.ascontiguousarray(
                    np.asarray(query)[b, qsl].T.astype(bf)
                ),
                "xk_t": np.ascontiguousarray(np.asarray(key)[b].T.astype(bf)),
                "xv_t": np.ascontiguousarray(np.asarray(value)[b].T.astype(bf)),
                "wq_t": wq_t,
                "wk_t": wk_t,
                "wv_t": wv_t,
                "wo_t": wo_t,
                "wb_cols": wb_cols,
                "ident": ident,
                "mask_t": np.ascontiguousarray(m2[qsl, :].T).astype(bf),
            }
        )
    return in_maps


def kernel(**inputs):
    from concourse.bass_utils import run_bass_kernel_spmd

    np_inputs = {k: np.asarray(v) for k, v in inputs.items()}
    in_maps = _prep(**np_inputs)
    nc = _get_nc()
    res = run_bass_kernel_spmd(nc, in_maps, list(range(NCORES)))
    out = np.empty((B, S, D), np.float32)
    for c in range(NCORES):
        b, half = c // 2, c % 2
        out[b, half * NQ:(half + 1) * NQ, :] = (
            res.results[c]["out_t"].astype(np.float32).T
        )
    return out


# revision 32
# speedup vs baseline: 1.0936x; 1.0771x over previous
"""Multi-head attention TRN2 kernel, 8-core SPMD, head-pair pipelined.

Sharding: each core owns 1024 query rows (batch b = core//2, sequence half
core%2) and computes the full forward pass for those rows. No collectives.

Single-core structure: the 16 heads are processed as 8 head-pairs. For each
pair the kernel projects K/V (full 2048-key sequence) and Q (1024 rows) for
just that pair's 128 features, then runs attention for its two heads. The
projection matmuls of pair hp+1 are injected into the attention kc-loop of
pair hp, so the tensor engine's projection work fills the gaps left while the
scalar engine (exp, the per-core floor at ~266us) processes score chunks.

All matmul operands are bf16 (f32 PSUM accumulation). Score chunks are
[128 k, 1024 q] in PSUM; exp runs on the scalar engine (scale 1/8, no max
subtraction -- scores/8 ~ N(0,1)), writing bf16; the 0/1 mask multiplies on
the vector engine (all-bf16 SBUF operands hit the DVE 4x perf mode). PV runs
in flipped orientation -- lhsT = exp-chunk slice [128 k, 128 q], rhs = V
chunk [128 k, 65] (65th column of ones accumulates the softmax denominator)
-- producing X.T accumulators [128 q, 65] at full PE rate. Normalization is
then a per-partition scalar multiply; a PE transpose (identity matmul) turns
the normalized [128 q, 128 d] tiles back into [128 d, 1024 q] slabs for the
output projection, which runs as a tail phase.

PSUM budget (8 banks): a shared 2-deep ring of [128,1024]-f32 slots carries
score chunks AND projection partials (4 banks); two 2-deep rings of
[128,4,128]-f32 tiles carry the per-head PV accumulators (4 banks); the
transpose output borrows a PV-ring slot between heads.
"""

from contextlib import ExitStack

import numpy as np

B, S, D, H, DH = 4, 2048, 1024, 16, 64
NQ = 1024          # query rows per core
NK = 2048          # keys per core (full sequence of its batch)
NCORES = 8
NP = 8             # head pairs

_CACHE = {}


def _build():
    import concourse.mybir as mybir
    import concourse.tile as tile
    from concourse import bacc

    f32 = mybir.dt.float32
    bf16 = mybir.dt.bfloat16
    EXP = mybir.ActivationFunctionType.Exp

    nc = bacc.Bacc(
        "TRN2",
        target_bir_lowering=False,
        debug=False,
        enable_asserts=False,
        num_devices=NCORES,
    )

    xq_d = nc.dram_tensor("xq_t", [D, NQ], bf16, kind="ExternalInput").ap()
    xk_d = nc.dram_tensor("xk_t", [D, NK], bf16, kind="ExternalInput").ap()
    xv_d = nc.dram_tensor("xv_t", [D, NK], bf16, kind="ExternalInput").ap()
    wq_d = nc.dram_tensor("wq_t", [NP, D, 128], bf16, kind="ExternalInput").ap()
    wk_d = nc.dram_tensor("wk_t", [NP, D, 128], bf16, kind="ExternalInput").ap()
    wv_d = nc.dram_tensor("wv_t", [NP, D, 128], bf16, kind="ExternalInput").ap()
    wo_d = nc.dram_tensor("wo_t", [8, 8, 128, 128], bf16, kind="ExternalInput").ap()
    mk_d = nc.dram_tensor("mask_t", [NK, NQ], bf16, kind="ExternalInput").ap()
    id_d = nc.dram_tensor("ident", [128, 128], bf16, kind="ExternalInput").ap()
    wbc_d = nc.dram_tensor("wb_cols", [128, 24], f32, kind="ExternalInput").ap()
    out_d = nc.dram_tensor("out_t", [D, NQ], bf16, kind="ExternalOutput").ap()

    def split8(ap_2d):
        # [1024, N] dram view -> [128, 8, N]: partition p, chunk i, col
        return ap_2d.rearrange("(i p) q -> p i q", p=128)

    with tile.TileContext(nc) as tc:
        stk = ExitStack()

        kpool = stk.enter_context(tc.tile_pool(name="konst", bufs=1))
        wbc = kpool.tile([128, 24], f32, name="wbc")
        nc.sync.dma_start(wbc[:], wbc_d[:, :])
        ident = kpool.tile([128, 128], bf16, name="ident")
        nc.sync.dma_start(ident[:], id_d[:, :])

        rpool = stk.enter_context(tc.tile_pool(name="resid", bufs=1))
        xk_sb = rpool.tile([128, 8, NK], bf16, name="xk_sb")
        xq_sb = rpool.tile([128, 8, NQ], bf16, name="xq_sb")
        xv_sb = rpool.tile([128, 8, NK], bf16, name="xv_sb")
        mask_sb = rpool.tile([128, 16, NQ], bf16, name="mask_sb")
        x_all = rpool.tile([128, 8, 8, 128], bf16, name="x_all")

        astk = ExitStack()
        wpool = astk.enter_context(tc.tile_pool(name="wring", bufs=2))
        kqpool = astk.enter_context(tc.tile_pool(name="kqring", bufs=2))
        vpool = astk.enter_context(tc.tile_pool(name="vring", bufs=2))
        pepool = astk.enter_context(tc.tile_pool(name="pering", bufs=16))
        xnpool = astk.enter_context(tc.tile_pool(name="xnring", bufs=2))
        rcpool = astk.enter_context(tc.tile_pool(name="rcring", bufs=2))
        stps = astk.enter_context(tc.tile_pool(name="stps", bufs=2, space="PSUM"))
        xtps = astk.enter_context(tc.tile_pool(name="xtps", bufs=2, space="PSUM"))

        slabs = {}

        def make_units(hp):
            """Projection work for pair hp as a list of small closures; each
            emits ~0.5us of PE work (plus its DMA / consumer ops)."""
            state = {
                "k": kqpool.tile([128, NK], bf16, tag="k", name="k_slab"),
                "q": kqpool.tile([128, NQ], bf16, tag="q", name="q_slab"),
                "v": vpool.tile([128, 16, 2, 65], bf16, tag="v", name="v_slab"),
            }

            def dma_wv():
                w = wpool.tile([128, 8, 128], bf16, tag="wv", name="wv_sb")
                nc.sync.dma_start(w[:], split8(wv_d[hp, :, :]))
                state["wv"] = w
                nc.gpsimd.memset(state["v"][:, :, :, 64:65], 1.0)

            def v_unit(kb):
                def run():
                    ps = stps.tile([128, 128], f32, tag="ps", name="ps_v")
                    for i in range(8):
                        nc.tensor.matmul(
                            ps[:],
                            lhsT=xv_sb[:, i, kb * 128:(kb + 1) * 128],
                            rhs=state["wv"][:, i, :],
                            start=(i == 0),
                            stop=(i == 7),
                        )
                    nc.gpsimd.tensor_copy(
                        state["v"][:, kb, :, 0:64],
                        ps[:].rearrange("p (h e) -> p h e", e=64),
                    )
                return run

            def dma_wk():
                w = wpool.tile([128, 8, 128], bf16, tag="wk", name="wk_sb")
                nc.sync.dma_start(w[:], split8(wk_d[hp, :, :]))
                state["wk"] = w

            def k_unit(kq, half):
                def run():
                    if half == 0:
                        state[f"psk{kq}"] = stps.tile(
                            [128, 512], f32, tag="ps", name="ps_k"
                        )
                    ps = state[f"psk{kq}"]
                    for i in range(4 * half, 4 * half + 4):
                        nc.tensor.matmul(
                            ps[:],
                            lhsT=state["wk"][:, i, :],
                            rhs=xk_sb[:, i, kq * 512:(kq + 1) * 512],
                            start=(i == 0),
                            stop=(i == 7),
                        )
                    if half == 1:
                        nc.vector.tensor_scalar_add(
                            state["k"][:, kq * 512:(kq + 1) * 512],
                            ps[:],
                            wbc[:, 8 + hp:9 + hp],
                        )
                return run

            def dma_wq():
                w = wpool.tile([128, 8, 128], bf16, tag="wq", name="wq_sb")
                nc.sync.dma_start(w[:], split8(wq_d[hp, :, :]))
                state["wq"] = w

            def q_unit(qh, half):
                def run():
                    if half == 0:
                        state[f"psq{qh}"] = stps.tile(
                            [128, 512], f32, tag="ps", name="ps_q"
                        )
                    ps = state[f"psq{qh}"]
                    for i in range(4 * half, 4 * half + 4):
                        nc.tensor.matmul(
                            ps[:],
                            lhsT=state["wq"][:, i, :],
                            rhs=xq_sb[:, i, qh * 512:(qh + 1) * 512],
                            start=(i == 0),
                            stop=(i == 7),
                        )
                    if half == 1:
                        nc.vector.tensor_scalar_add(
                            state["q"][:, qh * 512:(qh + 1) * 512],
                            ps[:],
                            wbc[:, hp:hp + 1],
                        )
                return run

            units = [dma_wk]
            units += [k_unit(kq, half) for kq in range(4) for half in range(2)]
            units += [dma_wq]
            units += [q_unit(qh, half) for qh in range(2) for half in range(2)]
            units += [dma_wv]
            units += [v_unit(kb) for kb in range(16)]
            slabs[hp] = state
            return units

        # One global software pipeline over all 256 score chunks (16 heads x
        # 16 key-chunks): scores(c) / exp(c) / mask(c) stream ahead while
        # PV(c - LAG) trails, so head and pair boundaries never drain the
        # scalar engine. Projection units for pair hp+1 flow through the
        # injection queue, one per chunk slot.
        from collections import deque

        LAG = 14
        CH = H * 16

        # DMA transfers serialize on one global device in issue order, so
        # issue strictly by first-need: pair-0 weight slabs, then xk (K
        # proj), xq, the first mask chunks (consumed progressively from the
        # first exp onward), then xv interleaved with the remaining mask.
        inject = deque()
        u0 = make_units(0)
        u0[0]()   # dma_wk
        u0[9]()   # dma_wq
        u0[14]()  # dma_wv + ones memset
        for i in range(8):
            nc.sync.dma_start(xk_sb[:, i, :], xk_d[i * 128:(i + 1) * 128, :])
        for i in range(8):
            nc.sync.dma_start(xq_sb[:, i, :], xq_d[i * 128:(i + 1) * 128, :])
        for i in range(4):
            nc.sync.dma_start(mask_sb[:, i, :], mk_d[i * 128:(i + 1) * 128, :])
        for i in range(8):
            nc.sync.dma_start(xv_sb[:, i, :], xv_d[i * 128:(i + 1) * 128, :])
            if 4 + i < 16:
                nc.sync.dma_start(
                    mask_sb[:, 4 + i, :], mk_d[(4 + i) * 128:(5 + i) * 128, :]
                )
        for i in range(12, 16):
            nc.sync.dma_start(mask_sb[:, i, :], mk_d[i * 128:(i + 1) * 128, :])
        for u in u0[1:9] + u0[10:14]:  # K*8, Q*4
            u()
        # Pad before pair 0's V units so the PE does not park on them while
        # the xv load is still in flight.
        inject.extend([None] * 10)
        inject.extend(u0[15:])  # V*16

        pe_hist = {}
        xts_of = {}
        xn_of = {}

        for c in range(CH + LAG):
            if c < CH:
                head, kc = c // 16, c % 16
                hp, hh = head // 2, head % 2
                if kc == 0 and hh == 0 and hp + 1 < NP:
                    inject.extend(make_units(hp + 1))
                ks = slabs[hp]["k"][hh * 64:(hh + 1) * 64, :]
                qs = slabs[hp]["q"][hh * 64:(hh + 1) * 64, :]
                st = stps.tile([128, NQ], f32, tag="st", name="st")
                for qh in range(2):
                    nc.tensor.matmul(
                        st[:, qh * 512:(qh + 1) * 512],
                        lhsT=ks[:, kc * 128:(kc + 1) * 128],
                        rhs=qs[:, qh * 512:(qh + 1) * 512],
                        start=True,
                        stop=True,
                    )
                if inject:
                    u = inject.popleft()
                    if u is not None:
                        u()
                pe = pepool.tile([128, NQ], bf16, tag="pe", name="pe")
                nc.scalar.activation(pe[:], st[:], EXP, scale=0.125)
                nc.vector.tensor_mul(pe[:], pe[:], mask_sb[:, kc, :])
                pe_hist[c] = pe
            if c >= LAG:
                cc = c - LAG
                head, kc = cc // 16, cc % 16
                hp, hh = head // 2, head % 2
                pe = pe_hist.pop(cc)
                if kc == 0:
                    xts_of[head] = (
                        xtps.tile([128, 4, 128], f32, tag="xt_lo",
                                  name="xt_lo", bufs=1),
                        xtps.tile([128, 4, 128], f32, tag="xt_hi",
                                  name="xt_hi", bufs=1),
                    )
                    if hh == 0:
                        xn_of[hp] = xnpool.tile(
                            [128, 8, 128], bf16, tag="xn", name="xn"
                        )
                xts = xts_of[head]
                for qc in range(8):
                    nc.tensor.matmul(
                        xts[qc // 4][:, qc % 4, 0:65],
                        lhsT=pe[:, qc * 128:(qc + 1) * 128],
                        rhs=slabs[hp]["v"][:, kc, hh, :],
                        start=(kc == 0),
                        stop=(kc == 15),
                    )
                if kc == 15:
                    xt_lo, xt_hi = xts_of.pop(head)
                    xn = xn_of[hp]
                    rc = rcpool.tile([128, 8, 1], f32, tag="rc", name="rc")
                    nc.vector.reciprocal(rc[:, 0:4, :], xt_lo[:, :, 64:65])
                    nc.vector.reciprocal(rc[:, 4:8, :], xt_hi[:, :, 64:65])
                    for qc in range(8):
                        nc.vector.tensor_scalar_mul(
                            xn[:, qc:qc + 1, hh * 64:(hh + 1) * 64],
                            (xt_lo, xt_hi)[qc // 4][
                                :, qc % 4:qc % 4 + 1, 0:64
                            ],
                            rc[:, qc:qc + 1, :],
                        )
                    if hh == 1:
                        xn_of.pop(hp)
                        tp = stps.tile([128, 8, 128], bf16, tag="ps", name="tp")
                        for qc in range(8):
                            nc.tensor.transpose(
                                tp[:, qc, :], xn[:, qc, :], ident[:]
                            )
                        nc.vector.tensor_copy(x_all[:, hp], tp[:])
        astk.close()

        # Tail: output projection out.T[f, q] = sum_d Wo.T[d, f] * X[d, q].
        with (
            tc.tile_pool(name="wo_ring", bufs=3) as wopool,
            tc.tile_pool(name="oc_ring", bufs=3) as ocpool,
            tc.tile_pool(name="ops", bufs=2, space="PSUM") as opool,
        ):
            for fb in range(8):
                wo_sb = wopool.tile([128, 8, 128], bf16, tag="wo", name="wo_sb")
                nc.sync.dma_start(
                    wo_sb[:], wo_d[fb].rearrange("i p f -> p i f")
                )
                for qh in range(2):
                    op = opool.tile([128, 512], f32, tag="op", name="op")
                    for dp in range(8):
                        nc.tensor.matmul(
                            op[:],
                            lhsT=wo_sb[:, dp, :],
                            rhs=x_all[:, dp, qh * 4:(qh + 1) * 4, :],
                            start=(dp == 0),
                            stop=(dp == 7),
                        )
                    outc = ocpool.tile([128, 512], bf16, tag="oc", name="outc")
                    nc.vector.tensor_scalar_add(
                        outc[:], op[:], wbc[:, 16 + fb:17 + fb]
                    )
                    nc.sync.dma_start(
                        out_d[fb * 128:(fb + 1) * 128, qh * 512:(qh + 1) * 512],
                        outc[:],
                    )
        stk.close()

    nc.compile()
    return nc


def _get_nc():
    if "nc" not in _CACHE:
        _CACHE["nc"] = _build()
    return _CACHE["nc"]


def _prep(query, key, value, mask, Wq, bq, Wk, bk, Wv, bv, Wo, bo):
    import ml_dtypes

    f = np.float32
    bf = ml_dtypes.bfloat16

    def wt_tiles(W):  # W [D, D] -> [8, D, 128] slices of W.T along fout
        WT = np.ascontiguousarray(np.asarray(W, dtype=f).T)
        return np.ascontiguousarray(
            np.stack([WT[:, i * 128:(i + 1) * 128] for i in range(8)], 0)
        ).astype(bf)

    wq_t = wt_tiles(Wq)
    wk_t = wt_tiles(Wk)
    wv_t = wt_tiles(Wv)
    WoT = np.ascontiguousarray(np.asarray(Wo, dtype=f).T)
    wo_t = np.ascontiguousarray(
        np.stack(
            [
                np.stack(
                    [
                        WoT[dp * 128:(dp + 1) * 128, fb * 128:(fb + 1) * 128]
                        for dp in range(8)
                    ],
                    0,
                )
                for fb in range(8)
            ],
            0,
        )
    ).astype(bf)
    bo_eff = (
        np.asarray(bo, dtype=np.float64)
        + np.asarray(Wo, dtype=np.float64) @ np.asarray(bv, dtype=np.float64)
    ).astype(f)
    wb_cols = np.stack(
        [np.asarray(b).astype(f).reshape(8, 128).T for b in (bq, bk, bo_eff)],
        1,
    ).reshape(128, 24)
    wb_cols = np.ascontiguousarray(wb_cols)
    ident = np.eye(128, dtype=f).astype(bf)
    m2 = np.asarray(mask)[0, 0]  # [S, S] int
    in_maps = []
    for c in range(NCORES):
        b, half = c // 2, c % 2
        qsl = slice(half * NQ, (half + 1) * NQ)
        in_maps.append(
            {
                "xq_t": np.ascontiguousarray(
                    np.asarray(query)[b, qsl].T.astype(bf)
                ),
                "xk_t": np.ascontiguousarray(np.asarray(key)[b].T.astype(bf)),
                "xv_t": np.ascontiguousarray(np.asarray(value)[b].T.astype(bf)),
                "wq_t": wq_t,
                "wk_t": wk_t,
                "wv_t": wv_t,
                "wo_t": wo_t,
                "wb_cols": wb_cols,
                "ident": ident,
                "mask_t": np.ascontiguousarray(m2[qsl, :].T).astype(bf),
            }
        )
    return in_maps


def kernel(**inputs):
    from concourse.bass_utils import run_bass_kernel_spmd

    np_inputs = {k: np.asarray(v) for k, v in inputs.items()}
    in_maps = _prep(**np_inputs)
    nc = _get_nc()
    res = run_bass_kernel_spmd(nc, in_maps, list(range(NCORES)))
    out = np.empty((B, S, D), np.float32)
    for c in range(NCORES):
        b, half = c // 2, c % 2
        out[b, half * NQ:(half + 1) * NQ, :] = (
            res.results[c]["out_t"].astype(np.float32).T
        )
    return out
